# revision 38
# baseline (speedup 1.0000x reference)
"""MoE (B=2,T=2048,D=768,E=8,K=2,H=1536) Trainium2 kernel.

Sparse expert-parallel over the 8 NeuronCores: the host computes the gate
(softmax + top-2) in numpy, gathers the tokens routed to each expert, and
core e runs expert e's FFN only on its ~B*T*K/E gathered tokens. The
per-token gate weight is applied on device; the host scatter-adds the two
weighted expert outputs per token.

Activations stay feature-major (x^T [D, tok]) so gate/up banks [D,H] and
the down bank [H,D] are already in the stationary-operand (lhsT) layout the
PE wants — no transposes on device. The big GEMMs run in float32r (the PE's
single-pass fp32 mode, ~3.4x the 4-pass fp32 rate; per-GEMM rel err ~1.5e-4).
"""

import numpy as np

import concourse.bass as bass
import concourse.mybir as mybir
import concourse.tile as tile
from concourse import bass_utils

# Problem shape (hardcoded per contract).
B, T, D, E, H, KTOP = 2, 2048, 768, 8, 1536, 2
NTOK = B * T            # 4096 tokens
TOK = 512               # max tokens per block
DC = D // 128           # 6 chunks of the D (contraction) dim
HC = H // 128           # 12 chunks of the H dim
F32 = mybir.dt.float32
F32R = mybir.dt.float32r


def _install_axon_ntff_hook():
    """Best-effort: register the antenv.axon_hooks NTFF profile hook that the
    agent image lacks, so trace=True (or BASS_TRACE=1) can profile under axon.
    Never raises."""
    try:
        import sys, types, contextlib, ctypes  # noqa: PLC0415
        import antenv  # noqa: PLC0415
        if "antenv.axon_hooks" in sys.modules:
            return
        _HOOK = [None]
        mod = types.ModuleType("antenv.axon_hooks")
        mod.set_axon_ntff_profile_hook = lambda h: _HOOK.__setitem__(0, h)
        mod.get_axon_ntff_profile_hook = lambda: _HOOK[0]
        sys.modules["antenv.axon_hooks"] = mod
        antenv.axon_hooks = mod

        lib = ctypes.CDLL("/opt/axon/libaxon_pjrt.so")
        if not hasattr(lib, "axon_start_nrt_profile"):
            return
        lib.axon_start_nrt_profile.argtypes = [
            ctypes.POINTER(ctypes.c_int64), ctypes.c_size_t]
        lib.axon_start_nrt_profile.restype = ctypes.c_int64
        lib.axon_stop_nrt_profile.argtypes = [ctypes.c_char_p]
        lib.axon_stop_nrt_profile.restype = ctypes.c_int64

        @contextlib.contextmanager
        def _hook(output_dir, device_ids):
            import jax  # noqa: PLC0415
            jax.devices()
            if device_ids:
                ids = (ctypes.c_int64 * len(device_ids))(*device_ids)
                rc = lib.axon_start_nrt_profile(ids, len(device_ids))
            else:
                rc = lib.axon_start_nrt_profile(None, 0)
            if rc != 0:
                raise RuntimeError(f"axon_start_nrt_profile rc={rc}")
            try:
                yield
            finally:
                lib.axon_stop_nrt_profile(str(output_dir).encode())

        mod.set_axon_ntff_profile_hook(_hook)
    except Exception:
        pass


def _split_multiwaits(nc):
    """This walrus build only supports one sync-wait per instruction; move
    extra waits onto preceding NOPs on the same engine."""
    for fn in nc.m.functions:
        for bb in fn.blocks:
            out = []
            for ins in bb.instructions:
                si = ins.sync_info
                if si is not None and si.on_wait is not None and len(si.on_wait) > 1:
                    waits = list(si.on_wait)
                    for i, w in enumerate(waits[:-1]):
                        out.append(mybir.InstNoOp(
                            name=f"{ins.name}-sw{i}",
                            engine=ins.engine,
                            sync_info=mybir.SyncInfo(on_wait=[w], on_update=[]),
                        ))
                    si.on_wait = [waits[-1]]
                    ins.sync_info = si
                out.append(ins)
            bb.instructions = out
    return nc


def build_nc(npad):
    """Expert FFN on `npad` gathered tokens (feature-major, f32r GEMMs)."""
    # Equal-ish blocks of at most TOK tokens (multiples of 128): balanced
    # blocks beat [512, 512, remainder] because per-block matmul count is
    # fixed while per-matmul cost scales with N.
    ntile = npad // 128
    nblk = -(-ntile // (TOK // 128))
    sizes = [(ntile // nblk + (1 if i < ntile % nblk else 0)) * 128
             for i in range(nblk)]
    blocks = []
    off = 0
    for s in sizes:
        blocks.append((off, s))
        off += s

    # All inputs are host-pre-swizzled to the exact SBUF layout so every DMA
    # is fully contiguous per partition (multi-KB lines -> peak DMA BW):
    #   xg_s[p, off*DC + c*tb + t] = x^T[c*128+p, off+t]       (block-major)
    #   gb_s[p, (ht*DC + c)*128 + j] = gate_bank[c*128+p, ht*128+j]
    #   db_s[p, (hk*DC + c)*128 + j] = down_bank[hk*128+p, c*128+j]
    # Output y_s uses the same block-major layout as xg_s.
    nc = bass.Bass()
    xg_s = nc.dram_tensor("xg_s", [128, DC * npad], F32R, kind="ExternalInput")
    gb_s = nc.dram_tensor("gb_s", [128, DC * H], F32R, kind="ExternalInput")
    ub_s = nc.dram_tensor("ub_s", [128, DC * H], F32R, kind="ExternalInput")
    db_s = nc.dram_tensor("db_s", [128, HC * D], F32R, kind="ExternalInput")
    wrow = nc.dram_tensor("wrow", [1, npad], F32R, kind="ExternalInput")
    onesd = nc.dram_tensor("onesd", [1, 128], F32R, kind="ExternalInput")
    y_s = nc.dram_tensor("y_s", [128, DC * npad], F32, kind="ExternalOutput")

    with tile.TileContext(nc) as tc:
        with (
            tc.tile_pool(name="wts", bufs=1) as wts,
            tc.tile_pool(name="xp", bufs=1) as xp,
            tc.tile_pool(name="hp", bufs=24) as hp,
            tc.tile_pool(name="sap", bufs=2) as sap,
            tc.tile_pool(name="yp", bufs=3) as yp,
            tc.tile_pool(name="wsp", bufs=2) as wsp,
            tc.tile_pool(name="ps", bufs=8, space="PSUM") as ps,
        ):
            # DMA plan. The SP HWDGE ring (strict FIFO) carries the
            # latency-critical first working set (ht0 slices of gb/ub,
            # block-0/1 x) and later the output chunks. The SWDGE (gpsimd)
            # queue streams the bank bulk; its transfers run CONCURRENTLY
            # with no priority, so they are dep-chained into paced groups
            # gated on the first working set. The ACT engine queue carries
            # no DMA at all (a dispatch stall there would delay every SILU
            # behind it).
            HTW = DC * 128                     # swizzled width of one ht slice
            NH2 = HC // 2
            gb0s = wts.tile([128, DC, 128], F32R)
            ub0s = wts.tile([128, DC, 128], F32R)
            gb12 = wts.tile([128, 2, DC, 128], F32R)
            ub12 = wts.tile([128, 2, DC, 128], F32R)
            gbA = wts.tile([128, NH2 - 3, DC, 128], F32R)
            ubA = wts.tile([128, NH2 - 3, DC, 128], F32R)
            gbB = wts.tile([128, NH2, DC, 128], F32R)
            ubB = wts.tile([128, NH2, DC, 128], F32R)
            db_h = [wts.tile([128, NH2, DC, 128], F32R, tag=f"dbh{i}",
                             name=f"dbh{i}") for i in range(2)]
            ones_sb = wts.tile([1, 128], F32R)
            wrow_sb = wts.tile([1, npad], F32R)

            xbs = [None] * nblk

            def emit_xb_dma(bi, ring):
                off, tb = blocks[bi]
                xbt = xp.tile([128, DC, tb], F32R,
                              tag=f"xb{'ab'[bi % 2]}", name=f"xb{bi}")
                ins = ring.dma_start(xbt[:], xg_s[:, off * DC:(off + tb) * DC])
                xbs[bi] = xbt
                return ins

            # First working set on the SP HWDGE ring (strict FIFO, gets the
            # full bandwidth while the SWDGE queue is still gated).
            d_gb0s = nc.sync.dma_start(gb0s[:], gb_s[:, 0:HTW])
            emit_xb_dma(0, nc.sync)
            nc.sync.dma_start(ub0s[:], ub_s[:, 0:HTW])
            nc.sync.dma_start(gb12[:], gb_s[:, HTW:3 * HTW])
            nc.sync.dma_start(ub12[:], ub_s[:, HTW:3 * HTW])
            if nblk > 1:
                emit_xb_dma(1, nc.sync)
            # Bulk weights on the SWDGE queue, chained one-after-another
            # (SWDGE transfers otherwise run concurrently with no priority,
            # which would make every tile land only when ALL of them land).
            # The chain head waits for the first working set.
            links = [
                [nc.gpsimd.dma_start(gbA[:], gb_s[:, 3 * HTW:NH2 * HTW]),
                 nc.gpsimd.dma_start(ubA[:], ub_s[:, 3 * HTW:NH2 * HTW])],
                [nc.gpsimd.dma_start(gbB[:], gb_s[:, NH2 * HTW:HC * HTW]),
                 nc.gpsimd.dma_start(ubB[:], ub_s[:, NH2 * HTW:HC * HTW])],
                [nc.gpsimd.dma_start(db_h[0][:], db_s[:, 0:NH2 * HTW]),
                 nc.gpsimd.dma_start(db_h[1][:], db_s[:, NH2 * HTW:HC * HTW]),
                 nc.gpsimd.dma_start(ones_sb[:], onesd[:]),
                 nc.gpsimd.dma_start(wrow_sb[:], wrow[:])],
            ]
            for a in links[0]:
                bass._add_dep_helper(a.ins, d_gb0s.ins, sync=True,
                                     reason="SWDGE flood waits for first set")
            for i in range(1, len(links)):
                for a in links[i]:
                    bass._add_dep_helper(a.ins, links[i - 1][0].ins,
                                         sync=True, reason="SWDGE chain")

            def gb_slice(ht, k):
                if ht == 0:
                    return gb0s[:, k, :]
                if ht < 3:
                    return gb12[:, ht - 1, k, :]
                if ht < NH2:
                    return gbA[:, ht - 3, k, :]
                return gbB[:, ht - NH2, k, :]

            def ub_slice(ht, k):
                if ht == 0:
                    return ub0s[:, k, :]
                if ht < 3:
                    return ub12[:, ht - 1, k, :]
                if ht < NH2:
                    return ubA[:, ht - 3, k, :]
                return ubB[:, ht - NH2, k, :]

            hts_all = {}

            def emit_g1(bi):
                # h = silu(x@gb) * (x@ub), feature-major [H, tb]
                xb = xbs[bi]
                tb = blocks[bi][1]
                hts = []
                for ht in range(HC):
                    a_ps = ps.tile([128, tb], F32, tag="ps")
                    for k in range(DC):
                        nc.tensor.matmul(a_ps[:], gb_slice(ht, k),
                                         xb[:, k, :],
                                         start=(k == 0), stop=(k == DC - 1))
                    u_ps = ps.tile([128, tb], F32, tag="ps")
                    for k in range(DC):
                        nc.tensor.matmul(u_ps[:], ub_slice(ht, k),
                                         xb[:, k, :],
                                         start=(k == 0), stop=(k == DC - 1))
                    sa = sap.tile([128, tb], F32)
                    nc.scalar.activation(sa[:], a_ps[:],
                                         mybir.ActivationFunctionType.Silu)
                    hch = hp.tile([128, tb], F32R, tag="h")
                    nc.vector.tensor_mul(hch[:], sa[:], u_ps[:])
                    hts.append(hch)
                hts_all[bi] = hts

            def emit_g2(bi):
                # y^T = db^T @ h, scaled by the per-token gate weight
                off, tb = blocks[bi]
                hts = hts_all.pop(bi)
                w_ps = ps.tile([128, tb], F32, tag="ps")
                nc.tensor.matmul(w_ps[:], ones_sb[:],
                                 wrow_sb[:, off:off + tb],
                                 start=True, stop=True)
                wsb = wsp.tile([128, tb], F32)
                nc.vector.tensor_copy(wsb[:], w_ps[:])
                for dt in range(DC):
                    y_ps = ps.tile([128, tb], F32, tag="ps")
                    for hk in range(HC):
                        half, kk = divmod(hk, NH2)
                        nc.tensor.matmul(y_ps[:], db_h[half][:, kk, dt, :],
                                         hts[hk][:],
                                         start=(hk == 0), stop=(hk == HC - 1))
                    ysb = yp.tile([128, tb], F32, tag="ysb")
                    nc.vector.tensor_mul(ysb[:], y_ps[:], wsb[:])
                    nc.sync.dma_start(
                        y_s[:, off * DC + dt * tb:
                            off * DC + (dt + 1) * tb], ysb[:])

            # Software-pipelined emission: the in-order PE always has the
            # next block's GEMM1 queued before this block's GEMM2, so DMA
            # pacing stalls in one never idle the other.
            emit_g1(0)
            if nblk > 1:
                emit_g1(1)
            for b in range(nblk):
                if b + 2 < nblk:
                    emit_xb_dma(b + 2, nc.sync)
                emit_g2(b)
                if b + 2 < nblk:
                    emit_g1(b + 2)

    return _split_multiwaits(nc)


_NC_CACHE = {}


def _routing(x2d, gate_w):
    """Replicates the reference gate: softmax over E, top-2, renormalize."""
    logits = x2d @ gate_w.T                                  # [NTOK, E] f32
    lmax = logits.max(-1, keepdims=True)
    p = np.exp(logits - lmax)
    p = p / p.sum(-1, keepdims=True)
    idx = np.argsort(-p, axis=-1, kind="stable")[:, :KTOP]   # [NTOK, 2]
    sel = np.take_along_axis(p, idx, -1)
    w = sel / (sel.sum(-1, keepdims=True) + 1e-8)            # [NTOK, 2]
    return idx, w.astype(np.float32)


def kernel(x, gate_w, gate_bank, up_bank, down_bank, _trace=False):
    _install_axon_ntff_hook()
    x = np.asarray(x, dtype=np.float32)
    gate_w = np.asarray(gate_w, dtype=np.float32)
    x2d = np.ascontiguousarray(x.reshape(NTOK, D))

    idx, w = _routing(x2d, gate_w)

    # Token lists per expert.
    tok_idx = []
    tok_w = []
    for e in range(E):
        hit = (idx == e)                        # [NTOK, 2]
        rows = np.nonzero(hit.any(-1))[0]
        tok_idx.append(rows)
        tok_w.append(w[rows, np.argmax(hit[rows], axis=-1)])
    nmax = max(len(r) for r in tok_idx)
    npad = ((nmax + 127) // 128) * 128

    key = npad
    if key not in _NC_CACHE:
        _NC_CACHE[key] = build_nc(npad)
    nc = _NC_CACHE[key]

    ntile = npad // 128
    nblk = -(-ntile // (TOK // 128))
    sizes = [(ntile // nblk + (1 if i < ntile % nblk else 0)) * 128
             for i in range(nblk)]
    offs = list(np.cumsum([0] + sizes[:-1]))

    def swz_x(xgT):
        out = np.empty((128, DC * npad), np.float32)
        for off, tb in zip(offs, sizes):
            seg = xgT[:, off:off + tb].reshape(DC, 128, tb)
            out[:, off * DC:(off + tb) * DC] = \
                seg.transpose(1, 0, 2).reshape(128, DC * tb)
        return out

    in_maps = []
    for e in range(E):
        rows = tok_idx[e]
        xg = np.zeros((npad, D), np.float32)
        xg[: len(rows)] = x2d[rows]
        wr = np.zeros((1, npad), np.float32)
        wr[0, : len(rows)] = tok_w[e]
        gbs = np.asarray(gate_bank[e], np.float32).reshape(DC, 128, HC, 128) \
            .transpose(1, 2, 0, 3).reshape(128, HC * DC * 128)
        ubs = np.asarray(up_bank[e], np.float32).reshape(DC, 128, HC, 128) \
            .transpose(1, 2, 0, 3).reshape(128, HC * DC * 128)
        dbs = np.asarray(down_bank[e], np.float32).reshape(HC, 128, DC, 128) \
            .transpose(1, 0, 2, 3).reshape(128, HC * DC * 128)
        in_maps.append({
            "xg_s": swz_x(np.ascontiguousarray(xg.T)),
            "gb_s": np.ascontiguousarray(gbs),
            "ub_s": np.ascontiguousarray(ubs),
            "db_s": np.ascontiguousarray(dbs),
            "wrow": wr,
            "onesd": np.ones((1, 128), np.float32),
        })

    res = bass_utils.run_bass_kernel_spmd(
        nc, in_maps, core_ids=list(range(8)), trace=_trace)

    y = np.zeros((NTOK, D), np.float32)
    for e in range(E):
        rows = tok_idx[e]
        ys = res.results[e]["y_s"]
        ygT = np.empty((D, npad), np.float32)
        for off, tb in zip(offs, sizes):
            ygT[:, off:off + tb] = ys[:, off * DC:(off + tb) * DC] \
                .reshape(128, DC, tb).transpose(1, 0, 2).reshape(D, tb)
        y[rows] += ygT[:, : len(rows)].T
    y = y.reshape(B, T, D)
    if _trace:
        return y, res
    return y
